# revision 1
# baseline (speedup 1.0000x reference)
"""Trainium2 Bass kernel for a 2-branch stacked-LSTM VAE encoder.

Reference computation (per batch row):
  z2_pre = stacked_lstm(x; z2 weights)           -> concat of final c states (2H)
  z2_mu / z2_logvar = dense heads; z2_sample = eps2*exp(0.5*lv)+mu
  z1_pre = stacked_lstm(concat(x, z2_sample broadcast over T); z1 weights)
  z1_mu / z1_logvar / z1_sample likewise.

Mapping: pure data parallelism over batch (16384 -> 8 cores x 2048).
On-chip layout is feature-major (features on partitions, batch on the free
dim) so LSTM weights are the stationary matmul operand and the recurrent
state streams as rhs.  Matmuls run as float32r (full PE rate), gate
nonlinearities on ScalarE (sigmoid/tanh share one table set), state updates
on VectorE.
"""

import numpy as np

T, F, ZD, H = 20, 80, 32, 256
B, NCORES = 16384, 8
BC = B // NCORES  # 2048 per core

_NC_CACHE = {}


def _build(bc, t_steps, reps=1):
    import concourse.mybir as mybir
    import concourse.tile as tile
    from concourse import bacc

    f32 = mybir.dt.float32
    f32r = mybir.dt.float32r
    AF = mybir.ActivationFunctionType

    ne = min(1024, bc)       # elementwise / ACT piece width (batch)
    nsub = min(512, bc)      # matmul moving-operand width
    n_halves = bc // ne
    nsubs = ne // nsub

    nc = bacc.Bacc("TRN2", target_bir_lowering=False, debug=False)
    dp = nc.declare_dram_parameter

    xT = dp("xT", (t_steps, F, bc), f32r, isOutput=False)
    eT = dp("eT", (ZD, 2, bc), f32, isOutput=False)  # [:, 0] eps1T, [:, 1] eps2T
    z2w1x = dp("z2w1x", (F, 1024), f32r, isOutput=False)
    z2w1h = dp("z2w1h", (2, 128, 1024), f32r, isOutput=False)
    z2w2 = dp("z2w2", (4, 128, 1024), f32r, isOutput=False)  # [U2_0,U2_1,W2_0,W2_1]
    z2b1 = dp("z2b1", (128, 8), f32, isOutput=False)
    z2b2 = dp("z2b2", (128, 8), f32, isOutput=False)
    z1w1x = dp("z1w1x", (ZD + F, 1024), f32r, isOutput=False)  # rows: z2s part, x part
    z1w1h = dp("z1w1h", (2, 128, 1024), f32r, isOutput=False)
    z1w2 = dp("z1w2", (4, 128, 1024), f32r, isOutput=False)
    z1b1 = dp("z1b1", (128, 8), f32, isOutput=False)
    z1b2 = dp("z1b2", (128, 8), f32, isOutput=False)
    hw2m = dp("hw2m", (4, 128, ZD), f32r, isOutput=False)
    hw2l = dp("hw2l", (4, 128, ZD), f32r, isOutput=False)
    hb2m = dp("hb2m", (ZD, 1), f32, isOutput=False)
    hb2l = dp("hb2l", (ZD, 1), f32, isOutput=False)
    hw1m = dp("hw1m", (4, 128, ZD), f32r, isOutput=False)
    hw1l = dp("hw1l", (4, 128, ZD), f32r, isOutput=False)
    hb1m = dp("hb1m", (ZD, 1), f32, isOutput=False)
    hb1l = dp("hb1l", (ZD, 1), f32, isOutput=False)
    out = dp("out", (6, ZD, bc), f32, isOutput=True)

    with tile.TileContext(nc) as tc:
        with (
            tc.tile_pool(name="wts", bufs=1) as wpool,
            tc.tile_pool(name="state", bufs=4) as spool,
            tc.tile_pool(name="gates", bufs=14) as gpool,
            tc.tile_pool(name="xin", bufs=2) as xpool,
            tc.tile_pool(name="xzp", bufs=1) as xzpool,
            tc.tile_pool(name="psum", bufs=4, space="PSUM") as ppool,
        ):
            def load_w_eager(w1x_d, b1_d, kx):
                # only what step 0 needs, so the first matmul starts ASAP
                w1x = wpool.tile([kx, 1024], f32r, tag="w1x")
                nc.sync.dma_start(out=w1x[:], in_=w1x_d[:])
                b1 = wpool.tile([128, 8], f32, tag="b1")
                nc.sync.dma_start(out=b1[:], in_=b1_d[:])
                return w1x, b1

            def load_w_rest(w1h_d, w2_d, b2_d):
                w1h = wpool.tile([128, 2, 1024], f32r, tag="w1h")
                for k in range(2):
                    nc.sync.dma_start(out=w1h[:, k, :], in_=w1h_d[k])
                w2 = wpool.tile([128, 4, 1024], f32r, tag="w2")
                for k in range(4):
                    nc.sync.dma_start(out=w2[:, k, :], in_=w2_d[k])
                b2 = wpool.tile([128, 8], f32, tag="b2")
                nc.sync.dma_start(out=b2[:], in_=b2_d[:])
                return w1h, w2, b2

            def make_states():
                # no memset needed: at t=0 every element of h/c is fully
                # written (emit_layer first=True path).
                sts = []
                for dt_ in (f32r, f32, f32r, f32):  # h1, c1, h2, c2
                    st = spool.tile([128, 2, bc], dt_, tag="state", name="state")
                    sts.append(st)
                return sts

            def emit_group(spec, m, n, gate):
                """One (m, n) PSUM accumulation group + its gate activation."""
                kchunks, bias = spec[0], spec[1]
                nk = len(kchunks)
                ps = ppool.tile([128, ne], f32, tag="ps")
                for sub in range(nsubs):
                    bsl = slice(n * ne + sub * nsub, n * ne + (sub + 1) * nsub)
                    psl = slice(sub * nsub, (sub + 1) * nsub)
                    for ki, (wfn, rfn) in enumerate(kchunks):
                        nc.tensor.matmul(
                            ps[:, psl],
                            wfn(m),
                            rfn(bsl),
                            start=(ki == 0),
                            stop=(ki == nk - 1),
                        )
                g = gpool.tile([128, ne], f32, tag="g")
                func = AF.Tanh if m in (4, 5) else AF.Sigmoid
                nc.scalar.activation(g[:], ps[:], func, bias=bias[:, m : m + 1])
                gate[m] = g

            def emit_elem_c(spec, gate, n):
                """c update for one n-half; returns a finisher that emits the
                deferred tanh(c) + h write (or nothing when h is dead)."""
                kchunks, bias, h_st, c_st, first, skip_h, post_c = spec
                nsl = slice(n * ne, (n + 1) * ne)
                for kc in range(2):
                    c_ap = c_st[:, kc, nsl]
                    if first:
                        nc.vector.tensor_mul(c_ap, gate[0 + kc][:], gate[4 + kc][:])
                    else:
                        m1 = gpool.tile([128, ne], f32, tag="g")
                        nc.vector.tensor_mul(m1[:], gate[2 + kc][:], c_ap)
                        m2 = gpool.tile([128, ne], f32, tag="g")
                        nc.vector.tensor_mul(m2[:], gate[0 + kc][:], gate[4 + kc][:])
                        nc.vector.tensor_add(c_ap, m1[:], m2[:])
                if skip_h:
                    return None

                o_gates = (gate[6], gate[7])

                def finish():
                    for kc in range(2):
                        th = gpool.tile([128, ne], f32, tag="g")
                        nc.scalar.activation(th[:], c_st[:, kc, nsl], AF.Tanh)
                        nc.vector.tensor_mul(
                            h_st[:, kc, nsl], o_gates[kc][:], th[:]
                        )

                return finish

            def emit_block(spec, n, pending):
                """8 matmul groups + c update; the tanh/h tail of the previous
                block is emitted after this block's DVE chain so the ACT queue
                never head-of-line blocks on the DVE chain."""
                # the deferred tanh/h tail of an earlier block must land
                # before this block's groups in program order: those groups
                # may read the h it writes (always true when n_halves == 1).
                if pending:
                    pending.pop(0)()
                gate = {}
                for m in range(8):
                    emit_group(spec, m, n, gate)
                fin = emit_elem_c(spec, gate, n)
                post_c = spec[6]
                if post_c is not None:
                    post_c(n)
                if fin is not None:
                    pending.append(fin)

            def emit_superstep(specA, specB, pending):
                """SW-pipeline two independent layer evaluations: specA = l2 of
                step t-1 (PE-heavy window), specB = l1 of step t (ACT-heavy
                window), alternating per n-half so ACT/DVE debt from an l1
                block is absorbed by the adjacent l2 block's slack and PSUM
                recycling never stalls the PE."""
                for n in range(n_halves):
                    for spec in (specA, specB):
                        if spec is not None:
                            emit_block(spec, n, pending)

            def emit_lstm(weights, states, x_rhs_fn, cr, l1_first_chunks=None):
                """cr is filled with {(ci, kc, n): f32r tile} holding the
                final c states rounded for the head matmuls, emitted inline
                with the last step so the head phase doesn't serialize."""
                w1x, w1h, w2, b1, b2 = weights
                h1, c1, h2, c2 = states

                def make_round(ci, c_st):
                    def post_c(n):
                        nsl = slice(n * ne, (n + 1) * ne)
                        for kc in range(2):
                            p = gpool.tile([128, ne], f32r, tag="g", name="crnd")
                            nc.vector.tensor_copy(p[:], c_st[:, kc, nsl])
                            cr[(ci, kc, n)] = p
                    return post_c

                prev_l2 = None
                pending = []
                for t in range(t_steps):
                    first = t == 0
                    last = t == t_steps - 1
                    x_rhs = x_rhs_fn(t)
                    if first and l1_first_chunks is not None:
                        l1 = l1_first_chunks
                    else:
                        l1 = [
                            (lambda m, w=w1x: w[:, m * 128 : (m + 1) * 128], x_rhs),
                        ]
                        if not first:
                            l1 += [
                                (lambda m, w=w1h: w[:, 0, m * 128 : (m + 1) * 128],
                                 lambda s, st=h1: st[:, 0, s]),
                                (lambda m, w=w1h: w[:, 1, m * 128 : (m + 1) * 128],
                                 lambda s, st=h1: st[:, 1, s]),
                            ]
                    # l1's h on the last step still feeds l2; l2's final h is
                    # dead (heads read only c), so its tanh/h tail is skipped.
                    emit_superstep(
                        prev_l2,
                        (l1, b1, h1, c1, first, False,
                         make_round(0, c1) if last else None),
                        pending,
                    )
                    l2 = []
                    if not first:
                        l2 += [
                            (lambda m, w=w2: w[:, 0, m * 128 : (m + 1) * 128],
                             lambda s, st=h2: st[:, 0, s]),
                            (lambda m, w=w2: w[:, 1, m * 128 : (m + 1) * 128],
                             lambda s, st=h2: st[:, 1, s]),
                        ]
                    l2 += [
                        (lambda m, w=w2: w[:, 2, m * 128 : (m + 1) * 128],
                         lambda s, st=h1: st[:, 0, s]),
                        (lambda m, w=w2: w[:, 3, m * 128 : (m + 1) * 128],
                         lambda s, st=h1: st[:, 1, s]),
                    ]
                    prev_l2 = (l2, b2, h2, c2, first, last,
                               make_round(1, c2) if last else None)
                emit_superstep(prev_l2, None, pending)
                for fin in pending:
                    fin()

            def emit_heads(hwm_d, hwl_d, hbm_d, hbl_d, cr, eps_col, mu_i, lv_i,
                           s_i, s_dest):
                hwm = wpool.tile([128, 4, ZD], f32r, tag="hwm")
                hwl = wpool.tile([128, 4, ZD], f32r, tag="hwl")
                for k in range(4):
                    nc.sync.dma_start(out=hwm[:, k, :], in_=hwm_d[k])
                    nc.sync.dma_start(out=hwl[:, k, :], in_=hwl_d[k])
                hbm = wpool.tile([ZD, 1], f32, tag="hbm")
                nc.sync.dma_start(out=hbm[:], in_=hbm_d[:])
                hbl = wpool.tile([ZD, 1], f32, tag="hbl")
                nc.sync.dma_start(out=hbl[:], in_=hbl_d[:])
                for n in range(n_halves):
                    nsl = slice(n * ne, (n + 1) * ne)
                    res = {}
                    for which, hw_t in (("m", hwm), ("l", hwl)):
                        ps = ppool.tile([128, ne], f32, tag="ps")
                        for sub in range(nsubs):
                            psl = slice(sub * nsub, (sub + 1) * nsub)
                            for ki in range(4):
                                nc.tensor.matmul(
                                    ps[:ZD, psl],
                                    hw_t[:, ki, :],
                                    cr[(ki // 2, ki % 2, n)][:, psl],
                                    start=(ki == 0),
                                    stop=(ki == 3),
                                )
                        v = gpool.tile([ZD, ne], f32, tag="g")
                        bias = hbm if which == "m" else hbl
                        nc.scalar.activation(
                            v[:], ps[:ZD, :], AF.Identity, bias=bias[:, 0:1]
                        )
                        res[which] = v
                    ex = gpool.tile([ZD, ne], f32, tag="g")
                    nc.scalar.activation(ex[:], res["l"][:], AF.Exp, scale=0.5)
                    sm = gpool.tile([ZD, ne], f32, tag="g")
                    nc.vector.tensor_mul(sm[:], eps[:, eps_col, nsl], ex[:])
                    dst = s_dest(n, nsl)
                    nc.vector.tensor_add(dst, sm[:], res["m"][:])
                    nc.sync.dma_start(out=out[mu_i][:, nsl], in_=res["m"][:])
                    nc.sync.dma_start(out=out[lv_i][:, nsl], in_=res["l"][:])
                    nc.sync.dma_start(out=out[s_i][:, nsl], in_=dst)

            for _rep in range(reps):
                # ---- z2 branch ----
                z2_w1x, z2_b1 = load_w_eager(z2w1x, z2b1, F)
                x_cache = {}

                def z2_x_rhs(t):
                    if t not in x_cache:
                        xt = xpool.tile([F, bc], f32r, tag="x", name="xt")
                        nc.sync.dma_start(out=xt[:], in_=xT[t])
                        x_cache[t] = xt
                    xt = x_cache[t]
                    return lambda s, tt=xt: tt[:, s]

                z2_x_rhs(0)  # x(0) DMA queued right after the eager weights
                z2_w1h, z2_w2, z2_b2 = load_w_rest(z2w1h, z2w2, z2b2)
                z2w = (z2_w1x, z2_w1h, z2_w2, z2_b1, z2_b2)
                z2_states = make_states()
                z2_cr = {}
                emit_lstm(z2w, z2_states, z2_x_rhs, z2_cr)

                # xz tile: rows 0-31 z2_sample (from z2 heads), 32-111 x_t
                xz = xzpool.tile([ZD + F, bc], f32r, tag="xz")
                eps = wpool.tile([ZD, 2, bc], f32, tag="eps")
                nc.sync.dma_start(out=eps[:], in_=eT[:])
                # z1 step-0 inputs, prefetched so the x-part matmuls can
                # overlap the z2 head computation
                z1_w1xs = wpool.tile([F, 1024], f32r, tag="w1xs")
                nc.sync.dma_start(out=z1_w1xs[:], in_=z1w1x[ZD : ZD + F])
                z1_x0 = xpool.tile([F, bc], f32r, tag="x", name="xt")
                nc.sync.dma_start(out=z1_x0[:], in_=xT[0])

                z2s_tiles = {}

                def z2_s_dest(n, nsl):
                    t = gpool.tile([ZD, ne], f32, tag="g")
                    z2s_tiles[n] = (t, nsl)
                    return t[:, :]

                emit_heads(hw2m, hw2l, hb2m, hb2l, z2_cr, 1, 3, 4, 5,
                           z2_s_dest)
                for n, (t, nsl) in z2s_tiles.items():
                    nc.vector.tensor_copy(xz[:ZD, nsl], t[:])

                # ---- z1 branch ----
                z1_w1x, z1_b1 = load_w_eager(z1w1x, z1b1, ZD + F)
                z1_w1h, z1_w2, z1_b2 = load_w_rest(z1w1h, z1w2, z1b2)
                z1w = (z1_w1x, z1_w1h, z1_w2, z1_b1, z1_b2)
                z1_states = make_states()
                z1_cr = {}

                z1_l1_first = [
                    (lambda m: z1_w1xs[:, m * 128 : (m + 1) * 128],
                     lambda s: z1_x0[:, s]),
                    (lambda m: z1_w1x[:ZD, m * 128 : (m + 1) * 128],
                     lambda s: xz[:ZD, s]),
                ]

                def z1_x_rhs(t):
                    if t > 0:
                        nc.sync.dma_start(out=xz[ZD : ZD + F, :], in_=xT[t])
                    return lambda s: xz[:, s]

                emit_lstm(z1w, z1_states, z1_x_rhs, z1_cr,
                          l1_first_chunks=z1_l1_first)

                def z1_s_dest(n, nsl):
                    t = gpool.tile([ZD, ne], f32, tag="g")
                    return t[:, :]

                emit_heads(hw1m, hw1l, hb1m, hb1l, z1_cr, 0, 0, 1, 2,
                           z1_s_dest)

    nc.finalize()
    return nc


def _get_nc(bc=BC, t_steps=T):
    key = (bc, t_steps)
    if key not in _NC_CACHE:
        _NC_CACHE[key] = _build(bc, t_steps)
    return _NC_CACHE[key]


def _pack_weights(i):
    """i: dict of the full-size input arrays. Returns name->array (replicated)."""
    f = np.float32

    def pk(a):
        return np.ascontiguousarray(a, dtype=f)

    return {
        "z2w1x": pk(i["z2_W1"]),
        "z2w1h": pk(i["z2_U1"].reshape(2, 128, 1024)),
        "z2w2": pk(
            np.stack(
                [i["z2_U2"][0:128], i["z2_U2"][128:256],
                 i["z2_W2"][0:128], i["z2_W2"][128:256]]
            )
        ),
        "z2b1": pk(i["z2_b1"].reshape(8, 128).T),
        "z2b2": pk(i["z2_b2"].reshape(8, 128).T),
        "z1w1x": pk(np.concatenate([i["z1_W1"][F : F + ZD], i["z1_W1"][0:F]], axis=0)),
        "z1w1h": pk(i["z1_U1"].reshape(2, 128, 1024)),
        "z1w2": pk(
            np.stack(
                [i["z1_U2"][0:128], i["z1_U2"][128:256],
                 i["z1_W2"][0:128], i["z1_W2"][128:256]]
            )
        ),
        "z1b1": pk(i["z1_b1"].reshape(8, 128).T),
        "z1b2": pk(i["z1_b2"].reshape(8, 128).T),
        "hw2m": pk(i["mu2_W"].reshape(4, 128, ZD)),
        "hw2l": pk(i["lv2_W"].reshape(4, 128, ZD)),
        "hb2m": pk(i["mu2_b"].reshape(ZD, 1)),
        "hb2l": pk(i["lv2_b"].reshape(ZD, 1)),
        "hw1m": pk(i["mu1_W"].reshape(4, 128, ZD)),
        "hw1l": pk(i["lv1_W"].reshape(4, 128, ZD)),
        "hb1m": pk(i["mu1_b"].reshape(ZD, 1)),
        "hb1l": pk(i["lv1_b"].reshape(ZD, 1)),
    }


def _run(inputs, trace=False):
    from concourse.bass_utils import run_bass_kernel_spmd

    nc = _get_nc()
    w = _pack_weights(inputs)
    x = np.asarray(inputs["x"], dtype=np.float32)
    e1 = np.asarray(inputs["eps1"], dtype=np.float32)
    e2 = np.asarray(inputs["eps2"], dtype=np.float32)

    in_maps = []
    for c in range(NCORES):
        sl = slice(c * BC, (c + 1) * BC)
        m = dict(w)
        m["xT"] = np.ascontiguousarray(x[sl].transpose(1, 2, 0))
        m["eT"] = np.ascontiguousarray(np.stack([e1[sl].T, e2[sl].T], axis=1))
        in_maps.append(m)

    r = run_bass_kernel_spmd(nc, in_maps, list(range(NCORES)), trace=trace)
    outs = [r.results[c]["out"] for c in range(NCORES)]
    full = np.concatenate(outs, axis=2)  # (6, 32, B)
    ret = tuple(np.ascontiguousarray(full[k].T) for k in range(6))
    return ret, r


def kernel(**inputs):
    try:
        ret, _ = _run(inputs, trace=False)
    except Exception:
        # transient device/runtime failure: rebuild once and retry
        _NC_CACHE.clear()
        ret, _ = _run(inputs, trace=False)
    return ret



# revision 2
# speedup vs baseline: 699.7987x; 699.7987x over previous
"""Trainium2 Bass kernel for a 2-branch stacked-LSTM VAE encoder (v4b).

v2 changes vs v1:
  - all matmul operands bf16 (weights, x, h, c); PSUM stays f32.
  - ACT engine runs sigmoid-only: tanh(x) = 2*sigmoid(2x) - 1, with the
    affine fixup on DVE (tensor_scalar mult/add).  This avoids the
    1283ns activation-table reload on every sigmoid<->tanh switch (the
    two functions live in different HW table sets).
  - all elementwise tiles bf16 -> DVE 2x_1p mode (0.5 cycles/elem).
  - matmul emission is chunk-outer / sub-inner so each stationary
    weight load serves nsubs consecutive matmuls.
  - dense heads read the final bf16 c states directly as matmul rhs.
  - dead gates skipped: f gates at t=0, o gates of the last l2 step.
"""

import numpy as np

T, F, ZD, H = 20, 80, 32, 256
B, NCORES = 16384, 8
BC = B // NCORES  # 2048 per core

_NC_CACHE = {}


def _build(bc, t_steps, reps=1):
    import concourse.mybir as mybir
    import concourse.tile as tile
    from concourse import bacc

    f32 = mybir.dt.float32
    bf16 = mybir.dt.bfloat16
    AF = mybir.ActivationFunctionType
    MUL = mybir.AluOpType.mult
    ADD = mybir.AluOpType.add

    ne = min(1024, bc)       # elementwise / ACT piece width (batch)
    nsub = min(512, bc)      # matmul moving-operand width
    n_halves = bc // ne
    nsubs = ne // nsub

    nc = bacc.Bacc("TRN2", target_bir_lowering=False, debug=False)
    dp = nc.declare_dram_parameter

    xT = dp("xT", (t_steps, F, bc), bf16, isOutput=False)
    eT = dp("eT", (ZD, 2, bc), f32, isOutput=False)  # [:, 0] eps1T, [:, 1] eps2T
    z2w1x = dp("z2w1x", (F, 1024), bf16, isOutput=False)
    z2w1h = dp("z2w1h", (2, 128, 1024), bf16, isOutput=False)
    z2w2 = dp("z2w2", (4, 128, 1024), bf16, isOutput=False)  # [U2_0,U2_1,W2_0,W2_1]
    z2b1 = dp("z2b1", (128, 8), f32, isOutput=False)
    z2b2 = dp("z2b2", (128, 8), f32, isOutput=False)
    z1w1x = dp("z1w1x", (ZD + F, 1024), bf16, isOutput=False)  # rows: z2s part, x part
    z1w1h = dp("z1w1h", (2, 128, 1024), bf16, isOutput=False)
    z1w2 = dp("z1w2", (4, 128, 1024), bf16, isOutput=False)
    z1b1 = dp("z1b1", (128, 8), f32, isOutput=False)
    z1b2 = dp("z1b2", (128, 8), f32, isOutput=False)
    hw2m = dp("hw2m", (4, 128, ZD), bf16, isOutput=False)
    hw2l = dp("hw2l", (4, 128, ZD), bf16, isOutput=False)
    hb2m = dp("hb2m", (ZD, 1), f32, isOutput=False)
    hb2l = dp("hb2l", (ZD, 1), f32, isOutput=False)
    hw1m = dp("hw1m", (4, 128, ZD), bf16, isOutput=False)
    hw1l = dp("hw1l", (4, 128, ZD), bf16, isOutput=False)
    hb1m = dp("hb1m", (ZD, 1), f32, isOutput=False)
    hb1l = dp("hb1l", (ZD, 1), f32, isOutput=False)
    out = dp("out", (6, ZD, bc), f32, isOutput=True)

    with tile.TileContext(nc) as tc:
        with (
            tc.tile_pool(name="wts", bufs=1) as wpool,
            tc.tile_pool(name="state", bufs=4) as spool,
            tc.tile_pool(name="gates", bufs=22) as gpool,
            tc.tile_pool(name="xin", bufs=2) as xpool,
            tc.tile_pool(name="xzp", bufs=1) as xzpool,
            tc.tile_pool(name="psum", bufs=4, space="PSUM") as ppool,
        ):
            def load_w_eager(w1x_d, b1_d, kx):
                # only what step 0 needs, so the first matmul starts ASAP
                w1x = wpool.tile([kx, 1024], bf16, tag="w1x")
                nc.sync.dma_start(out=w1x[:], in_=w1x_d[:])
                b1 = wpool.tile([128, 8], f32, tag="b1")
                nc.sync.dma_start(out=b1[:], in_=b1_d[:])
                return w1x, b1

            def load_w_rest(w1h_d, w2_d, b2_d):
                w1h = wpool.tile([128, 2, 1024], bf16, tag="w1h")
                for k in range(2):
                    nc.sync.dma_start(out=w1h[:, k, :], in_=w1h_d[k])
                w2 = wpool.tile([128, 4, 1024], bf16, tag="w2")
                for k in range(4):
                    nc.sync.dma_start(out=w2[:, k, :], in_=w2_d[k])
                b2 = wpool.tile([128, 8], f32, tag="b2")
                nc.sync.dma_start(out=b2[:], in_=b2_d[:])
                return w1h, w2, b2

            def make_states():
                # no memset needed: at t=0 every element of h/c is fully
                # written (emit_layer first=True path).
                sts = []
                for _ in range(4):  # h1, c1, h2, c2
                    st = spool.tile([128, 2, bc], bf16, tag="state",
                                    name="state")
                    sts.append(st)
                return sts

            def emit_group(spec, m, n, gate):
                """One (m, n) PSUM accumulation group + its gate activation.
                chunk-outer / sub-inner: one stationary load per chunk."""
                kchunks, bias = spec[0], spec[1]
                nk = len(kchunks)
                ps = ppool.tile([128, ne], f32, tag="ps")
                for ki, (wfn, rfn) in enumerate(kchunks):
                    w_ap = wfn(m)
                    for sub in range(nsubs):
                        bsl = slice(n * ne + sub * nsub,
                                    n * ne + (sub + 1) * nsub)
                        psl = slice(sub * nsub, (sub + 1) * nsub)
                        nc.tensor.matmul(
                            ps[:, psl],
                            w_ap,
                            rfn(bsl),
                            start=(ki == 0),
                            stop=(ki == nk - 1),
                        )
                # gate order i,f,g,o; m 4,5 are the cell (tanh) gate,
                # computed as sigmoid(2x) here (bias column pre-doubled);
                # the 2y-1 fixup happens on DVE in emit_elem_c.  The sigmoid
                # stays f32 so the tanh value is only rounded to bf16 once
                # (rounding sigma first would put ~2^-9 absolute noise on
                # tanh values near zero).
                if m in (4, 5):
                    g = gpool.tile([128, ne], f32, tag="gf", bufs=8)
                    nc.scalar.activation(g[:], ps[:], AF.Sigmoid,
                                         bias=bias[:, m : m + 1], scale=2.0)
                else:
                    g = gpool.tile([128, ne], bf16, tag="g")
                    nc.scalar.activation(g[:], ps[:], AF.Sigmoid,
                                         bias=bias[:, m : m + 1])
                gate[m] = g

            def emit_elem_c(spec, gate, n):
                """c update for one n-half; returns a finisher that emits the
                deferred sigmoid/h write (or nothing when h is dead)."""
                kchunks, bias, h_st, c_st, first, skip_h, post_c = spec
                nsl = slice(n * ne, (n + 1) * ne)
                # m2 = tanh(zg) * i fused: (sigma_g*2 - 1) * i in one DVE op
                for kc in range(2):
                    c_ap = c_st[:, kc, nsl]
                    if first:
                        acc = gpool.tile([128, 1], f32, tag="acc", bufs=4)
                        nc.vector.affine_mul_reduce(
                            c_ap, acc[:], gate[4 + kc][:], gate[0 + kc][:],
                            2.0, -1.0)
                    else:
                        m1 = gpool.tile([128, ne], bf16, tag="g")
                        nc.vector.tensor_mul(m1[:], gate[2 + kc][:], c_ap)
                        m2 = gpool.tile([128, ne], bf16, tag="g")
                        acc = gpool.tile([128, 1], f32, tag="acc", bufs=4)
                        nc.vector.affine_mul_reduce(
                            m2[:], acc[:], gate[4 + kc][:], gate[0 + kc][:],
                            2.0, -1.0)
                        nc.vector.tensor_add(c_ap, m1[:], m2[:])
                if skip_h:
                    return None

                o_gates = (gate[6], gate[7])

                def finish():
                    for kc in range(2):
                        th = gpool.tile([128, ne], f32, tag="gf", bufs=8)
                        nc.scalar.activation(th[:], c_st[:, kc, nsl],
                                             AF.Sigmoid, scale=2.0)
                        acc = gpool.tile([128, 1], f32, tag="acc", bufs=4)
                        nc.vector.affine_mul_reduce(
                            h_st[:, kc, nsl], acc[:], th[:], o_gates[kc][:],
                            2.0, -1.0)

                return finish

            def emit_block(spec, n, pending):
                """matmul groups + c update; the tanh/h tail of the previous
                block is emitted after this block's DVE chain so the ACT queue
                never head-of-line blocks on the DVE chain."""
                if pending:
                    pending.pop(0)()
                kchunks, bias, h_st, c_st, first, skip_h, post_c = spec
                gate = {}
                for m in range(8):
                    if first and m in (2, 3):
                        continue  # f gate unused at t=0 (c_prev == 0)
                    if skip_h and m in (6, 7):
                        continue  # o gate unused when h is dead (last l2)
                    emit_group(spec, m, n, gate)
                fin = emit_elem_c(spec, gate, n)
                if post_c is not None:
                    post_c(n)
                if fin is not None:
                    pending.append(fin)

            def emit_superstep(specA, specB, pending):
                """SW-pipeline two independent layer evaluations: specA = l2 of
                step t-1 (PE-heavy window), specB = l1 of step t (ACT-heavy
                window), spec-outer so each deferred h finisher lands
                two blocks before its first consumer."""
                for spec in (specA, specB):
                    if spec is not None:
                        for n in range(n_halves):
                            emit_block(spec, n, pending)

            def emit_lstm(weights, states, x_rhs_fn, l1_first_chunks=None):
                w1x, w1h, w2, b1, b2 = weights
                h1, c1, h2, c2 = states

                prev_l2 = None
                pending = []
                for t in range(t_steps):
                    first = t == 0
                    last = t == t_steps - 1
                    x_rhs = x_rhs_fn(t)
                    if first and l1_first_chunks is not None:
                        l1 = l1_first_chunks
                    else:
                        l1 = [
                            (lambda m, w=w1x: w[:, m * 128 : (m + 1) * 128],
                             x_rhs),
                        ]
                        if not first:
                            l1 += [
                                (lambda m, w=w1h: w[:, 0, m * 128 : (m + 1) * 128],
                                 lambda s, st=h1: st[:, 0, s]),
                                (lambda m, w=w1h: w[:, 1, m * 128 : (m + 1) * 128],
                                 lambda s, st=h1: st[:, 1, s]),
                            ]
                    # l1's h on the last step still feeds l2; l2's final h is
                    # dead (heads read only c), so its tanh/h tail is skipped.
                    emit_superstep(
                        prev_l2,
                        (l1, b1, h1, c1, first, False, None),
                        pending,
                    )
                    l2 = []
                    if not first:
                        l2 += [
                            (lambda m, w=w2: w[:, 0, m * 128 : (m + 1) * 128],
                             lambda s, st=h2: st[:, 0, s]),
                            (lambda m, w=w2: w[:, 1, m * 128 : (m + 1) * 128],
                             lambda s, st=h2: st[:, 1, s]),
                        ]
                    l2 += [
                        (lambda m, w=w2: w[:, 2, m * 128 : (m + 1) * 128],
                         lambda s, st=h1: st[:, 0, s]),
                        (lambda m, w=w2: w[:, 3, m * 128 : (m + 1) * 128],
                         lambda s, st=h1: st[:, 1, s]),
                    ]
                    prev_l2 = (l2, b2, h2, c2, first, last, None)
                emit_superstep(prev_l2, None, pending)
                for fin in pending:
                    fin()

            def emit_heads(hwm_d, hwl_d, hbm_d, hbl_d, c_sts, eps_col,
                           mu_i, lv_i, s_i, s_dest):
                hwm = wpool.tile([128, 4, ZD], bf16, tag="hwm")
                hwl = wpool.tile([128, 4, ZD], bf16, tag="hwl")
                for k in range(4):
                    nc.sync.dma_start(out=hwm[:, k, :], in_=hwm_d[k])
                    nc.sync.dma_start(out=hwl[:, k, :], in_=hwl_d[k])
                hbm = wpool.tile([ZD, 1], f32, tag="hbm")
                nc.sync.dma_start(out=hbm[:], in_=hbm_d[:])
                hbl = wpool.tile([ZD, 1], f32, tag="hbl")
                nc.sync.dma_start(out=hbl[:], in_=hbl_d[:])
                c1_st, c2_st = c_sts
                for n in range(n_halves):
                    nsl = slice(n * ne, (n + 1) * ne)
                    res = {}
                    for which, hw_t in (("m", hwm), ("l", hwl)):
                        ps = ppool.tile([128, ne], f32, tag="ps")
                        for ki in range(4):
                            c_st = c1_st if ki < 2 else c2_st
                            kc = ki % 2
                            for sub in range(nsubs):
                                bsl = slice(n * ne + sub * nsub,
                                            n * ne + (sub + 1) * nsub)
                                psl = slice(sub * nsub, (sub + 1) * nsub)
                                nc.tensor.matmul(
                                    ps[:ZD, psl],
                                    hw_t[:, ki, :],
                                    c_st[:, kc, bsl],
                                    start=(ki == 0),
                                    stop=(ki == 3),
                                )
                        v = gpool.tile([ZD, ne], f32, tag="g")
                        bias = hbm if which == "m" else hbl
                        nc.scalar.activation(
                            v[:], ps[:ZD, :], AF.Identity, bias=bias[:, 0:1]
                        )
                        res[which] = v
                    ex = gpool.tile([ZD, ne], f32, tag="g")
                    nc.scalar.activation(ex[:], res["l"][:], AF.Exp, scale=0.5)
                    sm = gpool.tile([ZD, ne], f32, tag="g")
                    nc.vector.tensor_mul(sm[:], eps[:, eps_col, nsl], ex[:])
                    dst = s_dest(n, nsl)
                    nc.vector.tensor_add(dst, sm[:], res["m"][:])
                    nc.sync.dma_start(out=out[mu_i][:, nsl], in_=res["m"][:])
                    nc.sync.dma_start(out=out[lv_i][:, nsl], in_=res["l"][:])
                    nc.sync.dma_start(out=out[s_i][:, nsl], in_=dst)

            for _rep in range(reps):
                # ---- z2 branch ----
                z2_w1x, z2_b1 = load_w_eager(z2w1x, z2b1, F)
                x_cache = {}

                def z2_x_rhs(t):
                    if t not in x_cache:
                        xt = xpool.tile([F, bc], bf16, tag="x", name="xt")
                        nc.sync.dma_start(out=xt[:], in_=xT[t])
                        x_cache[t] = xt
                    xt = x_cache[t]
                    return lambda s, tt=xt: tt[:, s]

                z2_x_rhs(0)  # x(0) DMA queued right after the eager weights
                z2_w1h, z2_w2, z2_b2 = load_w_rest(z2w1h, z2w2, z2b2)
                z2w = (z2_w1x, z2_w1h, z2_w2, z2_b1, z2_b2)
                z2_states = make_states()
                emit_lstm(z2w, z2_states, z2_x_rhs)

                # xz tile: rows 0-31 z2_sample (from z2 heads), 32-111 x_t
                xz = xzpool.tile([ZD + F, bc], bf16, tag="xz")
                eps = wpool.tile([ZD, 2, bc], f32, tag="eps")
                nc.sync.dma_start(out=eps[:], in_=eT[:])
                # z1 step-0 inputs, prefetched so the x-part matmuls can
                # overlap the z2 head computation
                z1_w1xs = wpool.tile([F, 1024], bf16, tag="w1xs")
                nc.sync.dma_start(out=z1_w1xs[:], in_=z1w1x[ZD : ZD + F])
                z1_x0 = xpool.tile([F, bc], bf16, tag="x", name="xt")
                nc.sync.dma_start(out=z1_x0[:], in_=xT[0])

                z2s_tiles = {}

                def z2_s_dest(n, nsl):
                    t = gpool.tile([ZD, ne], f32, tag="g")
                    z2s_tiles[n] = (t, nsl)
                    return t[:, :]

                emit_heads(hw2m, hw2l, hb2m, hb2l,
                           (z2_states[1], z2_states[3]), 1, 3, 4, 5,
                           z2_s_dest)
                for n, (t, nsl) in z2s_tiles.items():
                    nc.vector.tensor_copy(xz[:ZD, nsl], t[:])

                # ---- z1 branch ----
                z1_w1x, z1_b1 = load_w_eager(z1w1x, z1b1, ZD + F)
                z1_w1h, z1_w2, z1_b2 = load_w_rest(z1w1h, z1w2, z1b2)
                z1w = (z1_w1x, z1_w1h, z1_w2, z1_b1, z1_b2)
                z1_states = make_states()

                z1_l1_first = [
                    (lambda m: z1_w1xs[:, m * 128 : (m + 1) * 128],
                     lambda s: z1_x0[:, s]),
                    (lambda m: z1_w1x[:ZD, m * 128 : (m + 1) * 128],
                     lambda s: xz[:ZD, s]),
                ]

                def z1_x_rhs(t):
                    if t > 0:
                        nc.sync.dma_start(out=xz[ZD : ZD + F, :], in_=xT[t])
                    return lambda s: xz[:, s]

                emit_lstm(z1w, z1_states, z1_x_rhs,
                          l1_first_chunks=z1_l1_first)

                def z1_s_dest(n, nsl):
                    t = gpool.tile([ZD, ne], f32, tag="g")
                    return t[:, :]

                emit_heads(hw1m, hw1l, hb1m, hb1l,
                           (z1_states[1], z1_states[3]), 0, 0, 1, 2,
                           z1_s_dest)

    nc.finalize()
    return nc


def _get_nc(bc=BC, t_steps=T):
    key = (bc, t_steps)
    if key not in _NC_CACHE:
        _NC_CACHE[key] = _build(bc, t_steps)
    return _NC_CACHE[key]


def _pack_weights(i):
    """i: dict of the full-size input arrays. Returns name->array (replicated)."""
    import ml_dtypes

    bf = ml_dtypes.bfloat16
    f = np.float32

    def pk(a):
        return np.ascontiguousarray(np.asarray(a, dtype=f).astype(bf))

    def pkb(b):
        # (4H,) bias -> (128, 8); double the cell-gate columns (m 4,5)
        # because the kernel computes sigmoid(2x) for them.
        v = np.asarray(b, dtype=f).reshape(8, 128).T.copy()
        v[:, 4] *= 2.0
        v[:, 5] *= 2.0
        return np.ascontiguousarray(v)

    return {
        "z2w1x": pk(i["z2_W1"]),
        "z2w1h": pk(np.asarray(i["z2_U1"]).reshape(2, 128, 1024)),
        "z2w2": pk(
            np.stack(
                [i["z2_U2"][0:128], i["z2_U2"][128:256],
                 i["z2_W2"][0:128], i["z2_W2"][128:256]]
            )
        ),
        "z2b1": pkb(i["z2_b1"]),
        "z2b2": pkb(i["z2_b2"]),
        "z1w1x": pk(np.concatenate(
            [i["z1_W1"][F : F + ZD], i["z1_W1"][0:F]], axis=0)),
        "z1w1h": pk(np.asarray(i["z1_U1"]).reshape(2, 128, 1024)),
        "z1w2": pk(
            np.stack(
                [i["z1_U2"][0:128], i["z1_U2"][128:256],
                 i["z1_W2"][0:128], i["z1_W2"][128:256]]
            )
        ),
        "z1b1": pkb(i["z1_b1"]),
        "z1b2": pkb(i["z1_b2"]),
        "hw2m": pk(np.asarray(i["mu2_W"]).reshape(4, 128, ZD)),
        "hw2l": pk(np.asarray(i["lv2_W"]).reshape(4, 128, ZD)),
        "hb2m": np.ascontiguousarray(
            np.asarray(i["mu2_b"], dtype=f).reshape(ZD, 1)),
        "hb2l": np.ascontiguousarray(
            np.asarray(i["lv2_b"], dtype=f).reshape(ZD, 1)),
        "hw1m": pk(np.asarray(i["mu1_W"]).reshape(4, 128, ZD)),
        "hw1l": pk(np.asarray(i["lv1_W"]).reshape(4, 128, ZD)),
        "hb1m": np.ascontiguousarray(
            np.asarray(i["mu1_b"], dtype=f).reshape(ZD, 1)),
        "hb1l": np.ascontiguousarray(
            np.asarray(i["lv1_b"], dtype=f).reshape(ZD, 1)),
    }


def _pack_core_inputs(inputs, bc=BC, n_cores=NCORES):
    import ml_dtypes

    bf = ml_dtypes.bfloat16
    w = _pack_weights(inputs)
    x = np.asarray(inputs["x"], dtype=np.float32).astype(bf)
    e1 = np.asarray(inputs["eps1"], dtype=np.float32)
    e2 = np.asarray(inputs["eps2"], dtype=np.float32)

    in_maps = []
    for c in range(n_cores):
        sl = slice(c * bc, (c + 1) * bc)
        m = dict(w)
        m["xT"] = np.ascontiguousarray(x[sl].transpose(1, 2, 0))
        m["eT"] = np.ascontiguousarray(np.stack([e1[sl].T, e2[sl].T], axis=1))
        in_maps.append(m)
    return in_maps


def _run(inputs, trace=False):
    from concourse.bass_utils import run_bass_kernel_spmd

    nc = _get_nc()
    in_maps = _pack_core_inputs(inputs)

    r = run_bass_kernel_spmd(nc, in_maps, list(range(NCORES)), trace=trace)
    outs = [r.results[c]["out"] for c in range(NCORES)]
    full = np.concatenate(outs, axis=2)  # (6, 32, B)
    ret = tuple(np.ascontiguousarray(full[k].T) for k in range(6))
    return ret, r


def kernel(**inputs):
    try:
        ret, _ = _run(inputs, trace=False)
    except Exception:
        # transient device/runtime failure: rebuild once and retry
        _NC_CACHE.clear()
        ret, _ = _run(inputs, trace=False)
    return ret
